# revision 1
# baseline (speedup 1.0000x reference)
"""Hadamard transform kernel for Trainium2 (8 NeuronCores, SPMD).

Problem: x (8192, 4096) fp32; apply a 128-point Hadamard transform to each
contiguous 128-element group of every row.  Equivalent to
    out = (x.reshape(-1, 128) @ M).reshape(8192, 4096)
where M is the 128x128 butterfly matrix (symmetric, entries +/- 2^-3.5).

Strategy per core (rows sharded 8 ways -> 1024 rows/core):
  - DMA a 128-row tile [128, 4096] to SBUF (rows on partitions).
  - For each 128-col group: PE-transpose the 128x128 block into PSUM
    (contraction dim must live on partitions), copy PSUM->SBUF,
    then matmul(lhsT=block^T, rhs=M) -> PSUM gives the transformed block
    back in natural orientation; copy PSUM->SBUF and DMA the tile out.
"""

import math

import numpy as np

import concourse.bass as bass
import concourse.tile as tile
from concourse import bacc, mybir
from concourse.bass import ts
from concourse.bass_utils import run_bass_kernel_spmd

N_CORES = 8
ROWS, COLS = 8192, 4096
R_CORE = ROWS // N_CORES  # 1024 rows per core
G = 128                   # hadamard group size
NG = COLS // G            # 32 groups per row
NT = R_CORE // 128        # 8 row-tiles per core
QUADS = NG // 4           # 4 groups (one PSUM bank) per quad


def _hadamard_matrix() -> np.ndarray:
    """M = butterfly(I_128): out_row = x_row @ M (M symmetric)."""
    x = np.eye(G, dtype=np.float64)[..., None]
    for _ in range(int(math.log2(G))):
        top = x[..., ::2, :] + x[..., 1::2, :]
        bot = x[..., ::2, :] - x[..., 1::2, :]
        x = np.concatenate((top, bot), axis=-1) * (0.5 ** 0.5)
    return np.ascontiguousarray(x.squeeze(-2).astype(np.float32))


def _build_module():
    nc = bacc.Bacc("TRN2", target_bir_lowering=False, debug=False)
    f32 = mybir.dt.float32
    x_d = nc.dram_tensor("x", [R_CORE, COLS], f32, kind="ExternalInput")
    h_d = nc.dram_tensor("hmat", [G, G], f32, kind="ExternalInput")
    i_d = nc.dram_tensor("ident", [G, G], f32, kind="ExternalInput")
    o_d = nc.dram_tensor("out", [R_CORE, COLS], f32, kind="ExternalOutput")

    with tile.TileContext(nc) as tc:
        with (
            tc.tile_pool(name="const", bufs=1) as cpool,
            tc.tile_pool(name="xin", bufs=6) as xpool,
            tc.tile_pool(name="tt", bufs=8) as tpool,
            tc.tile_pool(name="outb", bufs=6) as opool,
            tc.tile_pool(name="pst", bufs=4, space=bass.MemorySpace.PSUM) as pst,
            tc.tile_pool(name="psm", bufs=4, space=bass.MemorySpace.PSUM) as psm,
        ):
            # PE warmup: dummy transposes with no data deps so the PE's
            # HAM clock-gate opens during the initial DMA wait.
            wsb = cpool.tile([G, G], f32)
            nc.gpsimd.memset(wsb[:], 1.0)
            wp = pst.tile([G, G], f32, tag="pt")
            for _ in range(26):
                nc.tensor.transpose(wp[:], wsb[:], wsb[:])

            hm = cpool.tile([G, G], f32)
            idm = cpool.tile([G, G], f32)
            nc.sync.dma_start(hm[:], h_d[:])
            nc.sync.dma_start(idm[:], i_d[:])

            # chunked 128-row tiles; small leading / trailing chunks
            # shorten pipeline fill and drain.  input DMAs ride the
            # Sync HWDGE ring, output DMAs the Scalar ring: separate
            # sequencers, so a store waiting on compute never blocks
            # the issue of the next load.
            for t in range(NT):
                if t == 0:
                    splits = [1024, 2048, 1024]
                elif t == NT - 1:
                    splits = [1024, 2048, 512, 512]
                else:
                    splits = [2048, 2048]
                c0 = 0
                for cc in splits:
                    xt = xpool.tile([128, cc], f32, tag="xt")
                    nc.sync.dma_start(
                        xt[:], x_d[t * 128:(t + 1) * 128, c0:c0 + cc]
                    )
                    ot = opool.tile([128, cc], f32, tag="ot")
                    for q in range(cc // 512):
                        pt = pst.tile([128, 512], f32, tag="pt")
                        for j in range(4):
                            g = (c0 // G) + q * 4 + j
                            nc.tensor.transpose(
                                pt[:, ts(j, G)],
                                xt[:, ts(q * 4 + j, G)],
                                idm[:],
                            )
                        tt = tpool.tile([128, 512], f32)
                        nc.vector.tensor_copy(tt[:], pt[:])
                        pm = psm.tile([128, 512], f32)
                        for j in range(4):
                            nc.tensor.matmul(
                                pm[:, ts(j, G)], tt[:, ts(j, G)], hm[:]
                            )
                        nc.scalar.copy(ot[:, ts(q, 512)], pm[:])
                    nc.scalar.dma_start(
                        o_d[t * 128:(t + 1) * 128, c0:c0 + cc], ot[:]
                    )
                    c0 += cc

    nc.compile()
    return nc


_NC_CACHE = None


def kernel(x) -> np.ndarray:
    global _NC_CACHE
    x = np.ascontiguousarray(np.asarray(x, dtype=np.float32))
    assert x.shape == (ROWS, COLS)
    if _NC_CACHE is None:
        _NC_CACHE = _build_module()
    nc = _NC_CACHE

    hmat = _hadamard_matrix()
    ident = np.eye(G, dtype=np.float32)
    in_maps = [
        {
            "x": np.ascontiguousarray(x[c * R_CORE:(c + 1) * R_CORE]),
            "hmat": hmat,
            "ident": ident,
        }
        for c in range(N_CORES)
    ]
    res = run_bass_kernel_spmd(nc, in_maps, core_ids=list(range(N_CORES)))
    return np.concatenate([r["out"] for r in res.results], axis=0)



# revision 4
# speedup vs baseline: 1.7559x; 1.7559x over previous
"""Hadamard transform kernel for Trainium2 (8 NeuronCores, SPMD).

Problem: x (8192, 4096) fp32; apply a 128-point Hadamard transform to each
contiguous 128-element group of every row:
    out = (x.reshape(-1, 128) @ M).reshape(8192, 4096),  M = butterfly(I_128).

The tolerance (rel err < 2e-2) admits bf16 transport, which halves the HBM
traffic -- the binding resource (per-core DMA cap ~358 GB/s; fp32 needs
33.5 MB/core, bf16 16.8 MB/core).

Strategy per core (rows sharded 8 ways -> 1024 rows/core):
  - Host repacks the shard k-major into xk[k, g, r] = x[r, g*128 + k]
    (shape [128, 32*1024] bf16).  The contraction index k then lives on
    SBUF partitions, so each 128-group is a single matmul
    psum[m, r] = sum_k M[k, m] * xk[k, g*1024 + r]  -- no PE transpose.
  - Pipeline: 1 MB chunk DMA in (sync ring) -> 8 matmuls N=512 -> PSUM
    copied to bf16 SBUF (split DVE/ACT) -> 1 MB chunk DMA out (scalar ring).
  - Host unpacks ok[m, g, r] -> out[r, g*128+m] and casts back to fp32.
"""

import math

import numpy as np
import ml_dtypes

import concourse.bass as bass
import concourse.tile as tile
from concourse import bacc, mybir
from concourse.bass_utils import run_bass_kernel_spmd

N_CORES = 8
ROWS, COLS = 8192, 4096
R_CORE = ROWS // N_CORES  # 1024 rows per core
G = 128                   # hadamard group size
NG = COLS // G            # 32 groups per row
FREE = NG * R_CORE        # 32768 free elements per partition
CH = 4                    # groups per pipeline chunk (4 * 256 KB = 1 MB)
NCH = NG // CH            # 8 chunks

BF16 = ml_dtypes.bfloat16


def _hadamard_matrix() -> np.ndarray:
    """M = butterfly(I_128): out_row = x_row @ M (M symmetric)."""
    x = np.eye(G, dtype=np.float64)[..., None]
    for _ in range(int(math.log2(G))):
        top = x[..., ::2, :] + x[..., 1::2, :]
        bot = x[..., ::2, :] - x[..., 1::2, :]
        x = np.concatenate((top, bot), axis=-1) * (0.5 ** 0.5)
    return np.ascontiguousarray(x.squeeze(-2).astype(np.float32))


def _build_module():
    nc = bacc.Bacc("TRN2", target_bir_lowering=False, debug=False)
    f32 = mybir.dt.float32
    bf16 = mybir.dt.bfloat16
    x_d = nc.dram_tensor("x", [G, FREE], bf16, kind="ExternalInput")
    h_d = nc.dram_tensor("hmat", [G, G], bf16, kind="ExternalInput")
    o_d = nc.dram_tensor("out", [G, FREE], bf16, kind="ExternalOutput")

    with tile.TileContext(nc) as tc:
        with (
            tc.tile_pool(name="const", bufs=1) as cpool,
            tc.tile_pool(name="xin", bufs=3) as xpool,
            tc.tile_pool(name="outb", bufs=3) as opool,
            tc.tile_pool(name="psm", bufs=8, space=bass.MemorySpace.PSUM) as psm,
        ):
            hm = cpool.tile([G, G], bf16)
            nc.sync.dma_start(hm[:], h_d[:])

            # PE warmup: dummy matmuls so the HAM clock gate opens during
            # the initial DMA wait.
            for _ in range(8):
                wp = psm.tile([G, G], f32, tag="pm")
                nc.tensor.matmul(wp[:], hm[:], hm[:])

            cc = CH * R_CORE  # chunk free width (4096)
            ncopy = 0
            for q in range(NCH):
                xt = xpool.tile([G, cc], bf16, tag="xt")
                nc.sync.dma_start(xt[:], x_d[:, q * cc:(q + 1) * cc])
                ot = opool.tile([G, cc], bf16, tag="ot")
                for j in range(2 * CH):
                    pm = psm.tile([G, 512], f32, tag="pm")
                    nc.tensor.matmul(
                        pm[:], hm[:], xt[:, j * 512:(j + 1) * 512]
                    )
                    # split PSUM->SBUF copies 2:1 between DVE and ACT
                    if ncopy % 3 == 2:
                        nc.scalar.copy(ot[:, j * 512:(j + 1) * 512], pm[:])
                    else:
                        nc.vector.tensor_copy(
                            ot[:, j * 512:(j + 1) * 512], pm[:]
                        )
                    ncopy += 1
                nc.scalar.dma_start(o_d[:, q * cc:(q + 1) * cc], ot[:])

    nc.compile()
    return nc


_NC_CACHE = None


def _get_nc():
    global _NC_CACHE
    if _NC_CACHE is None:
        _NC_CACHE = _build_module()
    return _NC_CACHE


def _in_maps(x: np.ndarray) -> list:
    """Full fp32 input -> per-core input maps (k-major bf16 repack)."""
    xb = x.astype(BF16)
    hmat = _hadamard_matrix().astype(BF16)
    maps = []
    for c in range(N_CORES):
        shard = xb[c * R_CORE:(c + 1) * R_CORE]          # [1024, 4096]
        xk = np.ascontiguousarray(
            shard.reshape(R_CORE, NG, G).transpose(2, 1, 0)
        ).reshape(G, FREE)                                # [128, 32*1024]
        maps.append({"x": xk, "hmat": hmat})
    return maps


def _unpack(results: list) -> np.ndarray:
    out = np.empty((ROWS, COLS), dtype=np.float32)
    for c, r in enumerate(results):
        ok = np.asarray(r["out"]).astype(np.float32)      # [128, 32*1024]
        out[c * R_CORE:(c + 1) * R_CORE] = (
            ok.reshape(G, NG, R_CORE).transpose(2, 1, 0).reshape(R_CORE, COLS)
        )
    return out


def kernel(x) -> np.ndarray:
    x = np.ascontiguousarray(np.asarray(x, dtype=np.float32))
    assert x.shape == (ROWS, COLS)
    nc = _get_nc()
    res = run_bass_kernel_spmd(nc, _in_maps(x), core_ids=list(range(N_CORES)))
    return _unpack(res.results)
